# revision 15
# baseline (speedup 1.0000x reference)
"""Trainium2 Bass kernel for nn_KGather (sparse_attention gather+scale).

Reference computation:
    out[n, p, t, w, c] = r_weight[n, p, t] * k[n, r_idx[n, p, t], w, c]
with n=16, p2=49, topk=8, w2=64, ck=128 (all fp32; r_idx int).

Strategy (8 cores, data parallel over n, 2 batch elements per core):
  - Host side: fold the gather indices AND the routing weights into a
    block-diagonal scaled one-hot matrix per core:
        onehot[j, pt] = r_weight[n_l, p, t]  if j == n_l*49 + r_idx[n_l, p, t]
    with pt = (n_l*49 + p)*8 + t, j in [0, 98).
  - Device side (static program, data-independent):
        out_core[pt, wc] = sum_j onehot[j, pt] * k_core[j, wc]
    i.e. a dense matmul on the TensorEngine. All device-side data is
    fp16: the one-hot column has exactly one nonzero, so each output
    element is a single fp16*fp16 product accumulated in fp32 PSUM and
    rounded once to fp16 on the drain -> worst-case relative error
    ~3*2^-11 ~ 0.15%, far inside the 2e-2 gate. fp16 (vs fp32) makes the
    matmul 4x faster on PE and halves both HBM loads and stores.
  - PSUM tiles are drained to an SBUF stage by alternating between the
    two engines that can read PSUM (ACT / DVE), two PSUM banks per copy
    to amortize instruction overhead; stages are stored with large
    contiguous DMAs. (GPSIMD/Pool cannot access PSUM on this HW.)
  - Host upcasts the fp16 output back to fp32.

Traffic per core: load 1.76 MB + store 12.8 MB ~= 14.6 MB at ~400 GB/s
aggregate DMA -> ~37 us memory floor.
"""

import numpy as np

# Problem shape (hardcoded per contest rules).
N, P2, TOPK, W2, CK = 16, 49, 8, 64, 128
NCORES = 8
NB = N // NCORES          # batch elements per core = 2
ROWS = NB * P2            # contraction dim per core = 98
PT = NB * P2 * TOPK       # output windows per core = 784
WC = W2 * CK              # window elements = 8192
PT_CHUNK = 112            # 7 pt chunks of 112 (<=128 partitions)
WC_CHUNK = 512            # 16 wc chunks of 512 (one fp32 PSUM bank)

_PROGRAM_CACHE = {}


def _build_program(patch=True):
    """Build the (data-independent) per-core Bass program.

    patch=True applies _split_multi_waits (required for the HW compile;
    the JSON round-trip breaks CoreSim, so use patch=False for sim)."""
    import concourse.bass as bass
    import concourse.mybir as mybir
    import concourse.tile as tile

    nc = bass.Bass()
    # onehot and k_core are packed into one input ([98, 784+8192]) so the
    # whole load is ONE DMA -> one completion semaphore.
    f16 = mybir.dt.float16
    f32 = mybir.dt.float32
    koh_d = nc.dram_tensor("koh", [ROWS, PT + WC], f16, kind="ExternalInput")
    out_d = nc.dram_tensor("out_core", [PT, WC], f16, kind="ExternalOutput")

    n_cp = PT // PT_CHUNK
    n_cw = WC // WC_CHUNK

    with tile.TileContext(nc) as tc:
        with (
            tc.tile_pool(name="const", bufs=1) as cpool,
            tc.tile_pool(name="stage", bufs=4) as spool,
            tc.tile_pool(name="psum", bufs=4, space="PSUM") as ppool,
        ):
            koh_sb = cpool.tile([ROWS, PT + WC], f16)
            # Two-part load on the two distinct HWDGE queues (SP + ACT) so
            # each half completes on its own semaphore: the first chunk's
            # matmuls (k cols < 4096) start while the rest of k loads.
            # Part B overlaps part A by one column: the intentional WAW
            # dependency SEQUENCES B after A, so A gets the full DMA
            # bandwidth and the PE pipeline starts ~4us earlier (B's data
            # is only needed by the chunk's second half, which trails).
            half = PT + WC // 2
            nc.sync.dma_start(out=koh_sb[:, :half], in_=koh_d[:, :half])
            nc.scalar.dma_start(out=koh_sb[:, half - 1:],
                                in_=koh_d[:, half - 1:])

            for cp in range(n_cp):
                stage = spool.tile([PT_CHUNK, WC], f16)
                lhsT = koh_sb[:, cp * PT_CHUNK:(cp + 1) * PT_CHUNK]
                # 8 drain groups of 2 PSUM banks (1024 cols) each. ALL
                # drains of the stage's first half go to ACT and of the
                # second half to DVE, so each half-stage store depends on
                # exactly ONE engine semaphore (DMA instructions can carry
                # only one wait condition; multi-waits on DMAs race).
                for g in range(n_cw // 2):
                    ps = ppool.tile([PT_CHUNK, 2 * WC_CHUNK], f32,
                                    space="PSUM")
                    for h in range(2):
                        cw = 2 * g + h
                        rhs = koh_sb[:, PT + cw * WC_CHUNK:
                                     PT + (cw + 1) * WC_CHUNK]
                        nc.tensor.matmul(
                            ps[:, h * WC_CHUNK:(h + 1) * WC_CHUNK],
                            lhsT=lhsT, rhs=rhs, start=True, stop=True)
                    sl = slice(2 * g * WC_CHUNK, 2 * (g + 1) * WC_CHUNK)
                    if g < 4:
                        nc.scalar.copy(out=stage[:, sl], in_=ps[:])
                    else:
                        nc.vector.tensor_copy(out=stage[:, sl], in_=ps[:])
                    # Store each quarter of the stage as soon as its two
                    # drains are done, so DMA ramps up early, flows
                    # smoothly, and the tail after the last drain is only
                    # a quarter stage. Each store still depends on a
                    # single engine semaphore.
                    if g % 2 == 1:
                        rows = slice(cp * PT_CHUNK, (cp + 1) * PT_CHUNK)
                        csl = slice((g - 1) * 2 * WC_CHUNK,
                                    (g + 1) * 2 * WC_CHUNK)
                        nc.sync.dma_start(out=out_d[rows, csl],
                                          in_=stage[:, csl])
    if patch:
        _split_multi_waits(nc)
    return nc


def _split_multi_waits(nc):
    """This walrus build rejects >1 fused sync-wait per instruction
    ("Too many sync wait commands"). Tile's wait assigner happily fuses
    several. Rewrite the BIR: for any instruction with N>1 waits, emit
    N-1 standalone single-wait EventSemaphore instructions (same engine,
    immediately before it) and keep only the last wait fused."""
    import json
    from concourse import mybir

    j = json.loads(mybir.module_to_json_string(nc.m))
    uid = [0]
    for f in j["functions"]:
        for b in f["blocks"]:
            out = []
            for ins in b["instructions"]:
                sync = ins.get("sync_info") or {}
                waits = sync.get("on_wait") or []
                if len(waits) > 1:
                    for w in waits[:-1]:
                        uid[0] += 1
                        out.append({
                            "debug": ins.get("debug", 0),
                            "engine": ins["engine"],
                            "ins": [],
                            "name": f"wsplit-{uid[0]}-{ins['name']}",
                            "opcode": "EventSemaphore",
                            "outs": [],
                            "sync_info": {"on_update": [], "on_wait": [w]},
                        })
                    sync["on_wait"] = [waits[-1]]
                out.append(ins)
            b["instructions"] = out
    nc.m = mybir.parse(j)


def get_program():
    if "nc" not in _PROGRAM_CACHE:
        _PROGRAM_CACHE["nc"] = _build_program()
    return _PROGRAM_CACHE["nc"]


def build_in_maps(r_idx, r_weight, k):
    """Host-side sharding + preprocessing: per-core inputs for the program."""
    r_idx = np.asarray(r_idx).astype(np.int64)
    r_weight = np.asarray(r_weight).astype(np.float32)
    k = np.asarray(k).astype(np.float32)

    pt = np.arange(PT)
    n_l = pt // (P2 * TOPK)
    p = (pt // TOPK) % P2
    t = pt % TOPK

    in_maps = []
    for c in range(NCORES):
        n0 = c * NB
        idx = r_idx[n0:n0 + NB]
        wgt = r_weight[n0:n0 + NB]
        koh = np.zeros((ROWS, PT + WC), np.float16)
        rows = n_l * P2 + idx[n_l, p, t]
        koh[rows, pt] = wgt[n_l, p, t].astype(np.float16)
        koh[:, PT:] = k[n0:n0 + NB].reshape(ROWS, WC).astype(np.float16)
        in_maps.append({"koh": koh})
    return in_maps


def run_program(in_maps, trace=False, **kwargs):
    from concourse.bass_utils import run_bass_kernel_spmd
    return run_bass_kernel_spmd(get_program(), in_maps,
                                list(range(NCORES)), trace=trace, **kwargs)


def assemble_output(results):
    out = np.empty((N, P2, TOPK, W2, CK), np.float32)
    for c in range(NCORES):
        out[c * NB:(c + 1) * NB] = np.asarray(
            results[c]["out_core"], dtype=np.float32).reshape(
            NB, P2, TOPK, W2, CK)
    return out


def kernel(r_idx, r_weight, k):
    in_maps = build_in_maps(r_idx, r_weight, k)
    res = run_program(in_maps)
    return assemble_output(res.results)


# revision 17
# speedup vs baseline: 1.1392x; 1.1392x over previous
"""Trainium2 Bass kernel for nn_KGather (sparse_attention gather+scale).

Reference computation:
    out[n, p, t, w, c] = r_weight[n, p, t] * k[n, r_idx[n, p, t], w, c]
with n=16, p2=49, topk=8, w2=64, ck=128 (all fp32; r_idx int).

Strategy (8 cores, data parallel over n, 2 batch elements per core):
  - Host side: fold the gather indices AND the routing weights into a
    block-diagonal scaled one-hot matrix per core:
        onehot[j, pt] = r_weight[n_l, p, t]  if j == n_l*49 + r_idx[n_l, p, t]
    with pt = (n_l*49 + p)*8 + t, j in [0, 98).
  - Device side (static program, data-independent):
        out_core[pt, wc] = sum_j onehot[j, pt] * k_core[j, wc]
    i.e. a dense matmul on the TensorEngine. All device-side data is
    fp16: the one-hot column has exactly one nonzero, so each output
    element is a single fp16*fp16 product accumulated in fp32 PSUM and
    rounded once to fp16 on the drain -> worst-case relative error
    ~3*2^-11 ~ 0.15%, far inside the 2e-2 gate. fp16 (vs fp32) makes the
    matmul 4x faster on PE and halves both HBM loads and stores.
  - PSUM tiles are drained to an SBUF stage by alternating between the
    two engines that can read PSUM (ACT / DVE), two PSUM banks per copy
    to amortize instruction overhead; stages are stored with large
    contiguous DMAs. (GPSIMD/Pool cannot access PSUM on this HW.)
  - Host upcasts the fp16 output back to fp32.

Traffic per core: load 1.76 MB + store 12.8 MB ~= 14.6 MB at ~400 GB/s
aggregate DMA -> ~37 us memory floor.
"""

import numpy as np

# Problem shape (hardcoded per contest rules).
N, P2, TOPK, W2, CK = 16, 49, 8, 64, 128
NCORES = 8
NB = N // NCORES          # batch elements per core = 2
ROWS = NB * P2            # contraction dim per core = 98
PT = NB * P2 * TOPK       # output windows per core = 784
WC = W2 * CK              # window elements = 8192
PT_CHUNK = 112            # 7 pt chunks of 112 (<=128 partitions)
WC_CHUNK = 512            # 16 wc chunks of 512 (one fp32 PSUM bank)

_PROGRAM_CACHE = {}


def _build_program(patch=True):
    """Build the (data-independent) per-core Bass program.

    patch=True applies _split_multi_waits (required for the HW compile;
    the JSON round-trip breaks CoreSim, so use patch=False for sim)."""
    import concourse.bass as bass
    import concourse.mybir as mybir
    import concourse.tile as tile

    nc = bass.Bass()
    # onehot and k_core are packed into one input ([98, 784+8192]) so the
    # whole load is ONE DMA -> one completion semaphore.
    f16 = mybir.dt.float16
    f32 = mybir.dt.float32
    koh_d = nc.dram_tensor("koh", [ROWS, PT + WC], f16, kind="ExternalInput")
    out_d = nc.dram_tensor("out_core", [PT, WC], f16, kind="ExternalOutput")

    n_cp = PT // PT_CHUNK
    n_cw = WC // WC_CHUNK

    with tile.TileContext(nc) as tc:
        with (
            tc.tile_pool(name="const", bufs=1) as cpool,
            tc.tile_pool(name="stage", bufs=4) as spool,
            tc.tile_pool(name="psum", bufs=4, space="PSUM") as ppool,
        ):
            koh_sb = cpool.tile([ROWS, PT + WC], f16)
            # Two-part load on the two distinct HWDGE queues (SP + ACT) so
            # each half completes on its own semaphore: the first chunk's
            # matmuls (k cols < 4096) start while the rest of k loads.
            # Part B overlaps part A by one column: the intentional WAW
            # dependency SEQUENCES B after A, so A gets the full DMA
            # bandwidth and the PE pipeline starts ~4us earlier (B's data
            # is only needed by the chunk's second half, which trails).
            half = PT + WC // 2
            nc.sync.dma_start(out=koh_sb[:, :half], in_=koh_d[:, :half])
            nc.scalar.dma_start(out=koh_sb[:, half:], in_=koh_d[:, half:])

            for cp in range(n_cp):
                stage = spool.tile([PT_CHUNK, WC], f16)
                lhsT = koh_sb[:, cp * PT_CHUNK:(cp + 1) * PT_CHUNK]
                # 8 drain groups of 2 PSUM banks (1024 cols) each. ALL
                # drains of the stage's first half go to ACT and of the
                # second half to DVE, so each half-stage store depends on
                # exactly ONE engine semaphore (DMA instructions can carry
                # only one wait condition; multi-waits on DMAs race).
                for g in range(n_cw // 2):
                    ps = ppool.tile([PT_CHUNK, 2 * WC_CHUNK], f32,
                                    space="PSUM")
                    for h in range(2):
                        cw = 2 * g + h
                        rhs = koh_sb[:, PT + cw * WC_CHUNK:
                                     PT + (cw + 1) * WC_CHUNK]
                        nc.tensor.matmul(
                            ps[:, h * WC_CHUNK:(h + 1) * WC_CHUNK],
                            lhsT=lhsT, rhs=rhs, start=True, stop=True)
                    sl = slice(2 * g * WC_CHUNK, 2 * (g + 1) * WC_CHUNK)
                    if g % 2 == 0:
                        nc.scalar.copy(out=stage[:, sl], in_=ps[:])
                    else:
                        nc.vector.tensor_copy(out=stage[:, sl], in_=ps[:])
                    # Store each half of the stage as soon as its four
                    # drains are done (two per engine; the store DMA's two
                    # semaphore waits are handled by _split_multi_waits).
                    if g == 3 or g == 7:
                        rows = slice(cp * PT_CHUNK, (cp + 1) * PT_CHUNK)
                        csl = slice((g - 3) * 2 * WC_CHUNK,
                                    (g + 1) * 2 * WC_CHUNK)
                        nc.sync.dma_start(out=out_d[rows, csl],
                                          in_=stage[:, csl])
    if patch:
        _split_multi_waits(nc)
    return nc


def _split_multi_waits(nc):
    """This walrus build rejects >1 fused sync-wait per instruction
    ("Too many sync wait commands"). Tile's wait assigner happily fuses
    several. Rewrite the BIR: for any instruction with N>1 waits, emit
    N-1 standalone single-wait EventSemaphore instructions (same engine,
    immediately before it) and keep only the last wait fused."""
    import json
    from concourse import mybir

    j = json.loads(mybir.module_to_json_string(nc.m))
    uid = [0]
    for f in j["functions"]:
        for b in f["blocks"]:
            out = []
            for ins in b["instructions"]:
                sync = ins.get("sync_info") or {}
                waits = sync.get("on_wait") or []
                if len(waits) > 1:
                    for w in waits[:-1]:
                        uid[0] += 1
                        out.append({
                            "debug": ins.get("debug", 0),
                            "engine": ins["engine"],
                            "ins": [],
                            "name": f"wsplit-{uid[0]}-{ins['name']}",
                            "opcode": "EventSemaphore",
                            "outs": [],
                            "sync_info": {"on_update": [], "on_wait": [w]},
                        })
                    sync["on_wait"] = [waits[-1]]
                out.append(ins)
            b["instructions"] = out
    nc.m = mybir.parse(j)


def get_program():
    if "nc" not in _PROGRAM_CACHE:
        _PROGRAM_CACHE["nc"] = _build_program()
    return _PROGRAM_CACHE["nc"]


def build_in_maps(r_idx, r_weight, k):
    """Host-side sharding + preprocessing: per-core inputs for the program."""
    r_idx = np.asarray(r_idx).astype(np.int64)
    r_weight = np.asarray(r_weight).astype(np.float32)
    k = np.asarray(k).astype(np.float32)

    pt = np.arange(PT)
    n_l = pt // (P2 * TOPK)
    p = (pt // TOPK) % P2
    t = pt % TOPK

    in_maps = []
    for c in range(NCORES):
        n0 = c * NB
        idx = r_idx[n0:n0 + NB]
        wgt = r_weight[n0:n0 + NB]
        koh = np.zeros((ROWS, PT + WC), np.float16)
        rows = n_l * P2 + idx[n_l, p, t]
        koh[rows, pt] = wgt[n_l, p, t].astype(np.float16)
        koh[:, PT:] = k[n0:n0 + NB].reshape(ROWS, WC).astype(np.float16)
        in_maps.append({"koh": koh})
    return in_maps


def run_program(in_maps, trace=False, **kwargs):
    from concourse.bass_utils import run_bass_kernel_spmd
    return run_bass_kernel_spmd(get_program(), in_maps,
                                list(range(NCORES)), trace=trace, **kwargs)


def assemble_output(results):
    out = np.empty((N, P2, TOPK, W2, CK), np.float32)
    for c in range(NCORES):
        out[c * NB:(c + 1) * NB] = np.asarray(
            results[c]["out_core"], dtype=np.float32).reshape(
            NB, P2, TOPK, W2, CK)
    return out


def kernel(r_idx, r_weight, k):
    in_maps = build_in_maps(r_idx, r_weight, k)
    res = run_program(in_maps)
    return assemble_output(res.results)
